# revision 29
# baseline (speedup 1.0000x reference)
"""Graphormer layer (LocalSubgraphEncoder) Trainium2 Bass kernel, v2.

Sharding: node-parallel over 8 cores. Core i computes the full layer output
for query nodes [512*i, 512*i+512): all 8 heads of attention over all 4096
key nodes, edge-type bias, softmax, output projection, residual, LayerNorm.
No cross-core communication; host concatenates row slices.

v2 design (from perfetto analysis of v1: PE saturated by unpacked K=32
matmuls, GPSIMD dense local_scatter, STT stuck in 1x mode):
 - all matmuls bf16; 2-head row-packing for QK (tile_position row groups)
   and 2-head column-packing for PV / denominator matmuls.
 - scores layout S^T [keys(part), queries(free)]: softmax denominator z
   comes from a packed ones-vector matmul into a shared PSUM bank.
 - edge bias applied multiplicatively AFTER exp: P = exp(S) * F where
   F = exp(scattered bias) is precomputed DENSE on the host and streamed
   from HBM (33.5 MB/core) -> one 2x-mode DVE tensor_tensor per tile;
   GPSIMD does nothing.
 - ACT (ScalarE) does exclusively the exp drain PSUM->SBUF bf16 in
   [128,1024] tiles: the ~128 us floor every design shares.
 - biases fused into DVE copies (per-partition scalar AP) or rank-1 PE
   matmuls; LayerNorm scale/shift via outer-product matmuls.
"""
import os
import sys
import math
import numpy as np

sys.path.insert(0, "/opt/trn_rl_repo")
import ml_dtypes  # noqa: E402
from concourse import bacc, bass, mybir, tile  # noqa: E402
from concourse.bass_utils import run_bass_kernel_spmd  # noqa: E402

N, D, H, E, NT = 4096, 256, 8, 131072, 16
DH = D // H            # 32
NCORES = 8
Q = N // NCORES        # 512 query nodes per core
KB = 128               # key-node block (partition dim)
NKB = N // KB          # 32
NPAIR = NKB // 2       # 16 (two key-blocks per [128,1024] score tile)
LN_EPS = 1e-5
SCALE = 1.0 / math.sqrt(DH)

f32 = mybir.dt.float32
bf16 = mybir.dt.bfloat16
EXP = mybir.ActivationFunctionType.Exp
SQRT = mybir.ActivationFunctionType.Sqrt
ADD = mybir.AluOpType.add
MULT = mybir.AluOpType.mult
SUB = mybir.AluOpType.subtract

_PROG = {}
LAST_RESULTS = None

WARM_START = int(os.environ.get("WARM_START", "0"))
WARM_BOOST = int(os.environ.get("WARM_BOOST", "0"))
WARM_PERIOD = int(os.environ.get("WARM_PERIOD", "0"))
WARM_LEN = int(os.environ.get("WARM_LEN", "8"))


def build_program():
    nc = bacc.Bacc(
        "TRN2", target_bir_lowering=False, debug=False, num_devices=NCORES
    )

    def din(name, shape, dt):
        return nc.dram_tensor(name, shape, dt, kind="ExternalInput").ap()

    hT_d = din("hT", [D, N], bf16)          # (x + pos)^T
    xqT_d = din("xqT", [D, Q], f32)         # x^T core slice (residual)
    Wq_d = din("Wq", [D, D], bf16)
    Wk_d = din("Wk", [D, D], bf16)
    Wv_d = din("Wv", [D, D], bf16)
    Wo_d = din("Wo", [D, D], bf16)
    bq_d = din("bq", [D, 1], f32)
    bk_d = din("bk", [D, 1], f32)
    bo_d = din("bo", [D, 1], f32)
    bv_d = din("bv", [1, D], bf16)
    gm_d = din("gm", [1, D], bf16)          # gamma row
    bt_d = din("bt", [1, D], bf16)          # beta row
    e128_d = din("e128", [KB, KB], bf16)    # block-broadcast matrix
    # dense exp(bias): row = (mh, t, partition), col = (h4, j, q)
    F_d = din("F", [2 * NPAIR * KB, 4 * 2 * Q], bf16)
    outT = nc.dram_tensor("outT", [D, Q], f32, kind="ExternalOutput").ap()

    hqT_d = din("hqT", [D, Q], bf16)        # h^T core query slice

    with tile.TileContext(nc) as tc:
        from contextlib import ExitStack

        with ExitStack() as ctx:
            cpool = ctx.enter_context(tc.tile_pool(name="consts", bufs=1))

            def ctile(shape, dt, tag):
                return cpool.tile(shape, dt, tag=tag, name=tag)

            # persistent SBUF residents
            hT = [ctile([KB, N], bf16, f"hT{c}") for c in range(2)]
            hq = [ctile([KB, Q], bf16, f"hq{c}") for c in range(2)]
            xq = [ctile([KB, Q], f32, f"xq{c}") for c in range(2)]
            wq = [ctile([KB, D], bf16, f"wq{c}") for c in range(2)]
            wk = [ctile([KB, D], bf16, f"wk{c}") for c in range(2)]
            wv = [ctile([KB, D], bf16, f"wv{c}") for c in range(2)]
            wo = [ctile([KB, D], bf16, f"wo{c}") for c in range(2)]
            bq = [ctile([KB, 1], f32, f"bq{c}") for c in range(2)]
            bk = [ctile([KB, 1], f32, f"bk{c}") for c in range(2)]
            bo = [ctile([KB, 1], f32, f"bo{c}") for c in range(2)]
            bv_r = ctile([1, D], bf16, "bv_r")
            gm = ctile([1, D], bf16, "gm")
            bt = ctile([1, D], bf16, "bt")
            e128 = ctile([KB, KB], bf16, "e128")
            kT = [ctile([KB, N], bf16, f"kT{c}") for c in range(2)]
            qTb = [ctile([KB, Q], bf16, f"qTb{c}") for c in range(2)]
            # V with ones column: [key, kb, h, 32 dims + 1 one]
            vSB = ctile([KB, NKB, H, DH + 1], bf16, "vSB")
            attnT = [ctile([KB, Q], bf16, f"attnT{c}") for c in range(2)]
            ones_1x128 = ctile([1, KB], bf16, "o1x128")
            ones_128x1 = ctile([KB, 1], bf16, "o128x1")
            ones_1xQ = ctile([1, Q], bf16, "o1xQ")
            epsT = ctile([1, 1], f32, "epsT")

            # ---- loads ----
            for c in range(2):
                sl = slice(c * KB, (c + 1) * KB)
                nc.sync.dma_start(out=hT[c][:], in_=hT_d[sl, :])
                nc.sync.dma_start(out=hq[c][:], in_=hqT_d[sl, :])
                nc.sync.dma_start(out=xq[c][:], in_=xqT_d[sl, :])
                nc.sync.dma_start(out=wq[c][:], in_=Wq_d[sl, :])
                nc.sync.dma_start(out=wk[c][:], in_=Wk_d[sl, :])
                nc.sync.dma_start(out=wv[c][:], in_=Wv_d[sl, :])
                nc.sync.dma_start(out=wo[c][:], in_=Wo_d[sl, :])
                nc.sync.dma_start(out=bq[c][:], in_=bq_d[sl, :])
                nc.sync.dma_start(out=bk[c][:], in_=bk_d[sl, :])
                nc.sync.dma_start(out=bo[c][:], in_=bo_d[sl, :])
            nc.sync.dma_start(out=bv_r[:], in_=bv_d[:])
            nc.sync.dma_start(out=gm[:], in_=gm_d[:])
            nc.sync.dma_start(out=bt[:], in_=bt_d[:])
            nc.sync.dma_start(out=e128[:], in_=e128_d[:])
            nc.vector.memset(ones_1x128[:], 1.0)
            nc.vector.memset(ones_128x1[:], 1.0)
            nc.vector.memset(ones_1xQ[:], 1.0)
            nc.vector.memset(epsT[:], LN_EPS)

            # ---- projections (all bf16, biases fused) ----
            with tc.tile_pool(name="pps", bufs=3, space="PSUM") as pps:
                # Q^T [2][128, 512] head-major partitions
                for mh in range(2):
                    ps = pps.tile([KB, Q], f32, tag="proj", name="proj")
                    for kc in range(2):
                        nc.tensor.matmul(
                            ps[:], wq[kc][:, mh * KB:(mh + 1) * KB], hq[kc][:],
                            start=(kc == 0), stop=(kc == 1),
                        )
                    nc.vector.tensor_scalar(
                        qTb[mh][:], ps[:], bq[mh][:], None, ADD
                    )
                # K^T [2][128, 4096]
                for mh in range(2):
                    for s in range(8):
                        ssl = slice(s * Q, (s + 1) * Q)
                        ps = pps.tile([KB, Q], f32, tag="proj", name="proj")
                        for kc in range(2):
                            nc.tensor.matmul(
                                ps[:], wk[kc][:, mh * KB:(mh + 1) * KB],
                                hT[kc][:, ssl],
                                start=(kc == 0), stop=(kc == 1),
                            )
                        nc.vector.tensor_scalar(
                            kT[mh][:, ssl], ps[:], bk[mh][:], None, ADD
                        )
                # V node-major [128, kb, h, 33] + bias via rank-1
                nc.vector.memset(vSB[:, :, :, DH], 1.0)
                for kb_i in range(NKB):
                    ksl = slice(kb_i * KB, (kb_i + 1) * KB)
                    psv = pps.tile([KB, H, DH], f32, tag="projv", name="projv")
                    for kc in range(2):
                        nc.tensor.matmul(
                            psv[:, :, :], hT[kc][:, ksl], wv[kc][:],
                            start=(kc == 0), stop=False,
                        )
                    nc.tensor.matmul(
                        psv[:, :, :], ones_1x128[:], bv_r[:],
                        start=False, stop=True,
                    )
                    nc.vector.tensor_copy(vSB[:, kb_i, :, 0:DH], psv[:, :, :])

            # ---- attention ----
            with ExitStack() as actx:
                sps = actx.enter_context(
                    tc.tile_pool(name="sps", bufs=3, space="PSUM")
                )
                ops = actx.enter_context(
                    tc.tile_pool(name="ops", bufs=2, space="PSUM")
                )
                spool = actx.enter_context(tc.tile_pool(name="spool", bufs=4))
                fpool = actx.enter_context(tc.tile_pool(name="fpool", bufs=3))
                npool = actx.enter_context(tc.tile_pool(name="npool", bufs=2))

                for mh in range(2):
                    # one oacc tile per head pair: partitions 0:33 head A
                    # (32 dims + z), 64:97 head B.
                    oacc = [
                        ops.tile([KB, Q], f32, tag="oacc", name="oacc")
                        for _ in range(2)
                    ]

                    def issue_pv(g):
                        t, pr, pf = g
                        first = (t == 0)
                        last = (t == NPAIR - 1)
                        for j in range(2):
                            kb_i = 2 * t + j
                            for hp in range(2):
                                h4 = 2 * pr + hp
                                h = 4 * mh + h4
                                nc.tensor.matmul(
                                    oacc[pr][64 * hp:64 * hp + DH + 1, :],
                                    vSB[:, kb_i, h, :],
                                    pf[hp][:, j * Q:(j + 1) * Q],
                                    start=(first and j == 0),
                                    stop=(last and j == 1),
                                    tile_position=(0, 64 * hp),
                                    skip_group_check=True,
                                )

                    # HAM warm-up: dependency-free burst of matmuls into the
                    # oacc region; the first real PV starts with start=True
                    # so the garbage is overwritten.
                    for w in range(WARM_START):
                        nc.tensor.matmul(
                            oacc[0][0:DH + 1, :], vSB[:, 0, 0, :], qTb[mh][:],
                            start=True, stop=True,
                            tile_position=(0, 0), skip_group_check=True,
                        )
                    # software pipeline: PE alternates QK(g) / PV(g-1) so it
                    # never stalls on the exp+mul chain of the current group.
                    prev = None
                    for t in range(NPAIR):
                        if WARM_PERIOD and t > 0 and t % WARM_PERIOD == 0:
                            # periodic dense matmul burst to re-flip HAM
                            bt_ps = sps.tile([KB, 2 * Q], f32, tag="sg",
                                             name="warm")
                            for w in range(WARM_LEN):
                                nc.tensor.matmul(
                                    bt_ps[:, 0:Q],
                                    kT[mh][0:32, 0:KB], qTb[mh][0:32, :],
                                    start=True, stop=True,
                                    tile_position=(0, 0),
                                    skip_group_check=True,
                                )
                        # one 1 MiB F transfer covers (mh, t) x 4 heads
                        fbig = fpool.tile([KB, 8 * Q], bf16, tag="ft",
                                          name="ft")
                        row = (mh * NPAIR + t) * KB
                        nc.sync.dma_start(
                            out=fbig[:], in_=F_d[row:row + KB, :]
                        )
                        for pr in range(2):      # head pairs (2p, 2p+1)
                            # QK: 2-head row-packed, j-outer for packing runs
                            sg = [
                                sps.tile([KB, 2 * Q], f32, tag="sg", name="sg")
                                for _ in range(2)
                            ]
                            # density boosters: dummy weight loads keep the
                            # PE array streaming (HAM warm) without touching
                            # PSUM; each real matmul reloads its own weights.
                            for w in range(WARM_BOOST):
                                nc.tensor.ldweights(
                                    kT[mh][:, 0:KB], tile_position=(0, 0),
                                )
                            for j in range(2):
                                kb_i = 2 * t + j
                                ksl = slice(kb_i * KB, (kb_i + 1) * KB)
                                for hp in range(2):
                                    h4 = 2 * pr + hp
                                    psl = slice(32 * h4, 32 * h4 + 32)
                                    nc.tensor.matmul(
                                        sg[hp][:, j * Q:(j + 1) * Q],
                                        kT[mh][psl, ksl],
                                        qTb[mh][psl, :],
                                        start=True, stop=True,
                                        tile_position=(32 * h4, 0),
                                    )
                            pf = [None, None]
                            for hp in range(2):
                                h4 = 2 * pr + hp
                                # exp (ACT) PSUM -> SBUF bf16
                                p0 = spool.tile(
                                    [KB, 2 * Q], bf16, tag="p0", name="p0"
                                )
                                nc.scalar.activation(
                                    p0[:], sg[hp][:], EXP, scale=SCALE
                                )
                                pf[hp] = spool.tile(
                                    [KB, 2 * Q], bf16, tag="pf", name="pf"
                                )
                                nc.vector.tensor_mul(
                                    pf[hp][:], p0[:],
                                    fbig[:, h4 * 2 * Q:(h4 + 1) * 2 * Q],
                                )
                            if prev is not None:
                                issue_pv(prev)
                            prev = (t, pr, pf)
                    issue_pv(prev)
                    prev = None
                    # ---- normalize: attn = oacc_num * (1/z) broadcast ----
                    # z rows: oacc[pr] partition 64*hp+32; gather to rows
                    # {0,32,64,96} of a [128, Q] tile, batch reciprocal.
                    zsb = npool.tile([KB, Q], f32, tag="zsb", name="zsb")
                    nc.vector.memset(zsb[:], 1.0)
                    for h4 in range(4):
                        pr, hp = h4 >> 1, h4 & 1
                        nc.vector.tensor_copy(
                            zsb[32 * h4:32 * h4 + 1, :],
                            oacc[pr][64 * hp + 32:64 * hp + 33, :],
                        )
                    rz = npool.tile([KB, Q], f32, tag="rz", name="rz")
                    nc.vector.reciprocal_approx_fast(rz[:], zsb[:])
                    rzb = npool.tile([KB, Q], bf16, tag="rzb", name="rzb")
                    nc.vector.tensor_copy(rzb[:], rz[:])
                    zbp = sps.tile([KB, Q], f32, tag="sg", name="zbp")
                    nc.tensor.matmul(
                        zbp[:], e128[:], rzb[:], start=True, stop=True
                    )
                    zbs = npool.tile([KB, Q], f32, tag="zbs", name="zbs")
                    nc.vector.tensor_copy(zbs[:], zbp[:])
                    for h4 in range(4):
                        pr, hp = h4 >> 1, h4 & 1
                        nc.vector.tensor_mul(
                            attnT[mh][32 * h4:32 * h4 + 32, :],
                            oacc[pr][64 * hp:64 * hp + 32, :],
                            zbs[32 * h4:32 * h4 + 32, :],
                        )

            # ---- output projection + residual + LayerNorm ----
            with ExitStack() as ectx:
                rps = ectx.enter_context(
                    tc.tile_pool(name="rps", bufs=1, space="PSUM")
                )
                epool = ectx.enter_context(tc.tile_pool(name="epool", bufs=2))
                out2 = [
                    epool.tile([KB, Q], f32, tag=f"out2_{c}", name=f"out2_{c}")
                    for c in range(2)
                ]
                for c in range(2):
                    op_ps = rps.tile([KB, Q], f32, tag="oproj", name="oproj")
                    for mh in range(2):
                        nc.tensor.matmul(
                            op_ps[:],
                            wo[mh][:, c * KB:(c + 1) * KB],
                            attnT[mh][:],
                            start=(mh == 0), stop=(mh == 1),
                        )
                    # out2 = (psum + bo) + x
                    nc.vector.scalar_tensor_tensor(
                        out2[c][:], op_ps[:], bo[c][:], xq[c][:],
                        op0=ADD, op1=ADD,
                    )
                # stats: mu, s2 via ones matmuls (f32)
                ones_f = epool.tile([KB, 1], f32, tag="onesf", name="onesf")
                nc.vector.memset(ones_f[:], 1.0)
                mu_ps = rps.tile([1, Q], f32, tag="mu", name="mu")
                for c in range(2):
                    nc.tensor.matmul(
                        mu_ps[:], ones_f[:], out2[c][:],
                        start=(c == 0), stop=(c == 1),
                        skip_group_check=True,
                    )
                s2_ps = rps.tile([1, Q], f32, tag="s2", name="s2")
                for c in range(2):
                    sq = epool.tile([KB, Q], f32, tag="sq", name="sq")
                    nc.vector.tensor_mul(sq[:], out2[c][:], out2[c][:])
                    nc.tensor.matmul(
                        s2_ps[:], ones_f[:], sq[:],
                        start=(c == 0), stop=(c == 1),
                        skip_group_check=True,
                    )
                mu = epool.tile([1, Q], f32, tag="mu_s", name="mu_s")
                nc.vector.tensor_scalar_mul(mu[:], mu_ps[:], 1.0 / D)
                m2 = epool.tile([1, Q], f32, tag="m2", name="m2")
                nc.vector.tensor_mul(m2[:], mu[:], mu[:])
                var = epool.tile([1, Q], f32, tag="var", name="var")
                nc.vector.scalar_tensor_tensor(
                    var[:], s2_ps[:], 1.0 / D, m2[:], op0=MULT, op1=SUB,
                )
                sd = epool.tile([1, Q], f32, tag="sd", name="sd")
                nc.scalar.activation(sd[:], var[:], SQRT, bias=epsT[:])
                rstd = epool.tile([1, Q], f32, tag="rstd", name="rstd")
                nc.vector.reciprocal_approx_fast(rstd[:], sd[:])
                # broadcast tiles via outer products:
                # c1 = gamma (x) rstd ; c2 = beta (x) 1 - gamma (x) (mu*rstd)
                rstd_b = epool.tile([1, Q], bf16, tag="rstdb", name="rstdb")
                nc.vector.tensor_copy(rstd_b[:], rstd[:])
                mr = epool.tile([1, Q], f32, tag="mr", name="mr")
                nc.vector.tensor_mul(mr[:], mu[:], rstd[:])
                mrn = epool.tile([1, Q], bf16, tag="mrn", name="mrn")
                nc.vector.tensor_scalar_mul(mrn[:], mr[:], -1.0)
                for c in range(2):
                    csl = slice(c * KB, (c + 1) * KB)
                    c1p = rps.tile([KB, Q], f32, tag="c1", name="c1")
                    nc.tensor.matmul(
                        c1p[:], gm[:, csl], rstd_b[:], start=True, stop=True
                    )
                    # c2 = gamma (x) (-mu*rstd) + beta (x) 1
                    c2p = rps.tile([KB, Q], f32, tag="c2", name="c2")
                    nc.tensor.matmul(
                        c2p[:], gm[:, csl], mrn[:], start=True, stop=False
                    )
                    nc.tensor.matmul(
                        c2p[:], bt[:, csl], ones_1xQ[:],
                        start=False, stop=True,
                    )
                    t1 = epool.tile([KB, Q], f32, tag="t1", name="t1")
                    nc.vector.tensor_mul(t1[:], out2[c][:], c1p[:])
                    y = epool.tile([KB, Q], f32, tag="y", name="y")
                    nc.vector.tensor_add(y[:], t1[:], c2p[:])
                    nc.sync.dma_start(out=outT[csl, :], in_=y[:])

    nc.compile()
    return nc


def _prep_F(q_idx, k_idx, bias_eh):
    """Dense multiplicative bias F = exp(scattered bias), per core.

    Row-block order matches kernel consumption: [mh, t, h4, partition]."""
    key = q_idx.astype(np.int64) * N + k_idx.astype(np.int64)
    uk, inv = np.unique(key, return_inverse=True)
    acc = np.zeros((len(uk), H), np.float32)
    np.add.at(acc, inv, bias_eh)
    uq = (uk // N).astype(np.int32)
    ukey = (uk % N).astype(np.int32)
    vals16 = np.exp(acc).astype(ml_dtypes.bfloat16).view(np.uint16)

    Fs = []
    for i in range(NCORES):
        sel = (uq >> 9) == i
        q = uq[sel] & (Q - 1)
        k = ukey[sel]
        v = vals16[sel]
        t = k >> 8
        j = (k >> 7) & 1
        p = k & (KB - 1)
        col = j * Q + q
        F16 = np.full((2, NPAIR, KB, 4, 2 * Q), 0x3F80, np.uint16)
        for h in range(H):
            F16[h >> 2, t, p, h & 3, col] = v[:, h]
        Fs.append(
            np.ascontiguousarray(F16.reshape(2 * NPAIR * KB, 4 * 2 * Q))
            .view(ml_dtypes.bfloat16)
        )
    return Fs


def kernel(**inputs):
    global LAST_RESULTS, _PROG
    x = np.asarray(inputs["x"], np.float32)
    pos = np.asarray(inputs["pos_encoding"], np.float32)
    ei = np.asarray(inputs["edge_index"])
    et = np.asarray(inputs["edge_types"])
    emb = np.asarray(inputs["edge_emb"], np.float32)
    W = {k: np.asarray(inputs[k], np.float32) for k in ("Wq", "Wk", "Wv", "Wo")}
    b = {k: np.asarray(inputs[k], np.float32).reshape(-1)
         for k in ("bq", "bk", "bv", "bo", "gamma", "beta")}

    bias_eh = emb[et]  # [E, H]
    Fs = _prep_F(ei[0], ei[1], bias_eh)

    pkey = (WARM_START, WARM_BOOST, WARM_PERIOD, WARM_LEN)
    if pkey not in _PROG:
        _PROG[pkey] = build_program()
    nc = _PROG[pkey]

    h = (x + pos).astype(np.float32)
    hT = np.ascontiguousarray(h.T.astype(ml_dtypes.bfloat16))
    xT = np.ascontiguousarray(x.T)
    Wb = {k: np.ascontiguousarray(w.astype(ml_dtypes.bfloat16))
          for k, w in W.items()}
    col = lambda a: np.ascontiguousarray(a.reshape(D, 1))
    row16 = lambda a: np.ascontiguousarray(
        a.reshape(1, D).astype(ml_dtypes.bfloat16)
    )
    e128 = np.zeros((KB, KB), np.float32)
    for h4 in range(4):
        e128[32 * h4, 32 * h4:32 * h4 + 32] = 1.0
    e128 = np.ascontiguousarray(e128.astype(ml_dtypes.bfloat16))

    in_maps = []
    for i in range(NCORES):
        sl = slice(i * Q, (i + 1) * Q)
        in_maps.append({
            "hT": hT,
            "hqT": np.ascontiguousarray(hT[:, sl]),
            "xqT": np.ascontiguousarray(xT[:, sl]),
            "Wq": Wb["Wq"], "Wk": Wb["Wk"], "Wv": Wb["Wv"], "Wo": Wb["Wo"],
            "bq": col(b["bq"]), "bk": col(b["bk"]), "bo": col(b["bo"]),
            "bv": row16(b["bv"]), "gm": row16(b["gamma"]),
            "bt": row16(b["beta"]), "e128": e128,
            "F": Fs[i],
        })

    trace = os.environ.get("BASS_KERNEL_TRACE", "0") == "1"
    try:
        res = run_bass_kernel_spmd(
            nc, in_maps, list(range(NCORES)), trace=trace
        )
    except Exception:
        if not trace:
            raise
        res = run_bass_kernel_spmd(nc, in_maps, list(range(NCORES)))
    LAST_RESULTS = res

    out = np.empty((N, D), np.float32)
    for i in range(NCORES):
        out[i * Q:(i + 1) * Q, :] = np.asarray(
            res.results[i]["outT"], np.float32
        ).T
    return out


# revision 32
# speedup vs baseline: 1.3071x; 1.3071x over previous
"""Graphormer layer (LocalSubgraphEncoder) Trainium2 Bass kernel, v2.

Sharding: node-parallel over 8 cores. Core i computes the full layer output
for query nodes [512*i, 512*i+512): all 8 heads of attention over all 4096
key nodes, edge-type bias, softmax, output projection, residual, LayerNorm.
No cross-core communication; host concatenates row slices.

v2 design (from perfetto analysis of v1: PE saturated by unpacked K=32
matmuls, GPSIMD dense local_scatter, STT stuck in 1x mode):
 - all matmuls bf16; 2-head row-packing for QK (tile_position row groups)
   and 2-head column-packing for PV / denominator matmuls.
 - scores layout S^T [keys(part), queries(free)]: softmax denominator z
   comes from a packed ones-vector matmul into a shared PSUM bank.
 - edge bias applied multiplicatively AFTER exp: P = exp(S) * F where
   F = exp(scattered bias) is precomputed DENSE on the host and streamed
   from HBM (33.5 MB/core) -> one 2x-mode DVE tensor_tensor per tile;
   GPSIMD does nothing.
 - ACT (ScalarE) does exclusively the exp drain PSUM->SBUF bf16 in
   [128,1024] tiles: the ~128 us floor every design shares.
 - biases fused into DVE copies (per-partition scalar AP) or rank-1 PE
   matmuls; LayerNorm scale/shift via outer-product matmuls.
"""
import os
import sys
import math
import numpy as np

sys.path.insert(0, "/opt/trn_rl_repo")
import ml_dtypes  # noqa: E402
from concourse import bacc, bass, mybir, tile  # noqa: E402
from concourse.bass_utils import run_bass_kernel_spmd  # noqa: E402

N, D, H, E, NT = 4096, 256, 8, 131072, 16
DH = D // H            # 32
NCORES = 8
Q = N // NCORES        # 512 query nodes per core
KB = 128               # key-node block (partition dim)
NKB = N // KB          # 32
NPAIR = NKB // 2       # 16 (two key-blocks per [128,1024] score tile)
LN_EPS = 1e-5
SCALE = 1.0 / math.sqrt(DH)

f32 = mybir.dt.float32
bf16 = mybir.dt.bfloat16
EXP = mybir.ActivationFunctionType.Exp
SQRT = mybir.ActivationFunctionType.Sqrt
ADD = mybir.AluOpType.add
MULT = mybir.AluOpType.mult
SUB = mybir.AluOpType.subtract

_PROG = {}
LAST_RESULTS = None

WARM_START = int(os.environ.get("WARM_START", "0"))
WARM_BOOST = int(os.environ.get("WARM_BOOST", "0"))
WARM_PERIOD = int(os.environ.get("WARM_PERIOD", "0"))
WARM_LEN = int(os.environ.get("WARM_LEN", "8"))


def build_program():
    nc = bacc.Bacc(
        "TRN2", target_bir_lowering=False, debug=False, num_devices=NCORES
    )

    def din(name, shape, dt):
        return nc.dram_tensor(name, shape, dt, kind="ExternalInput").ap()

    hT_d = din("hT", [D, N], bf16)          # (x + pos)^T
    xqT_d = din("xqT", [D, Q], f32)         # x^T core slice (residual)
    Wq_d = din("Wq", [D, D], bf16)
    Wk_d = din("Wk", [D, D], bf16)
    Wv_d = din("Wv", [D, D], bf16)
    Wo_d = din("Wo", [D, D], bf16)
    bq_d = din("bq", [D, 1], f32)
    bk_d = din("bk", [D, 1], f32)
    bo_d = din("bo", [D, 1], f32)
    bv_d = din("bv", [1, D], bf16)
    gm_d = din("gm", [1, D], bf16)          # gamma row
    bt_d = din("bt", [1, D], bf16)          # beta row
    e128_d = din("e128", [KB, KB], bf16)    # block-broadcast matrix
    # dense exp(bias): row = (mh, t, partition), col = (h4, j, q)
    F_d = din("F", [2 * NPAIR * KB, 4 * 2 * Q], bf16)
    outT = nc.dram_tensor("outT", [D, Q], f32, kind="ExternalOutput").ap()

    hqT_d = din("hqT", [D, Q], bf16)        # h^T core query slice

    with tile.TileContext(nc) as tc:
        from contextlib import ExitStack

        with ExitStack() as ctx:
            cpool = ctx.enter_context(tc.tile_pool(name="consts", bufs=1))

            def ctile(shape, dt, tag):
                return cpool.tile(shape, dt, tag=tag, name=tag)

            # persistent SBUF residents
            hT = [ctile([KB, N], bf16, f"hT{c}") for c in range(2)]
            hq = [ctile([KB, Q], bf16, f"hq{c}") for c in range(2)]
            xq = [ctile([KB, Q], f32, f"xq{c}") for c in range(2)]
            wq = [ctile([KB, D], bf16, f"wq{c}") for c in range(2)]
            wk = [ctile([KB, D], bf16, f"wk{c}") for c in range(2)]
            wv = [ctile([KB, D], bf16, f"wv{c}") for c in range(2)]
            wo = [ctile([KB, D], bf16, f"wo{c}") for c in range(2)]
            bq = [ctile([KB, 1], f32, f"bq{c}") for c in range(2)]
            bk = [ctile([KB, 1], f32, f"bk{c}") for c in range(2)]
            bo = [ctile([KB, 1], f32, f"bo{c}") for c in range(2)]
            bv_r = ctile([1, D], bf16, "bv_r")
            gm = ctile([1, D], bf16, "gm")
            bt = ctile([1, D], bf16, "bt")
            e128 = ctile([KB, KB], bf16, "e128")
            kT = [ctile([KB, N], bf16, f"kT{c}") for c in range(2)]
            qTb = [ctile([KB, Q], bf16, f"qTb{c}") for c in range(2)]
            # V with ones column: [key, kb, h, 32 dims + 1 one]
            vSB = ctile([KB, NKB, H, DH + 1], bf16, "vSB")
            attnT = [ctile([KB, Q], bf16, f"attnT{c}") for c in range(2)]
            ones_1x128 = ctile([1, KB], bf16, "o1x128")
            ones_128x1 = ctile([KB, 1], bf16, "o128x1")
            ones_1xQ = ctile([1, Q], bf16, "o1xQ")
            epsT = ctile([1, 1], f32, "epsT")
            warmup_in = ctile([1, 32], f32, "warmup_in")
            warmup_out = ctile([1, 32], bf16, "warmup_out")

            # ---- loads ----
            for c in range(2):
                sl = slice(c * KB, (c + 1) * KB)
                nc.sync.dma_start(out=hT[c][:], in_=hT_d[sl, :])
                nc.sync.dma_start(out=hq[c][:], in_=hqT_d[sl, :])
                nc.sync.dma_start(out=xq[c][:], in_=xqT_d[sl, :])
                nc.sync.dma_start(out=wq[c][:], in_=Wq_d[sl, :])
                nc.sync.dma_start(out=wk[c][:], in_=Wk_d[sl, :])
                nc.sync.dma_start(out=wv[c][:], in_=Wv_d[sl, :])
                nc.sync.dma_start(out=wo[c][:], in_=Wo_d[sl, :])
                nc.sync.dma_start(out=bq[c][:], in_=bq_d[sl, :])
                nc.sync.dma_start(out=bk[c][:], in_=bk_d[sl, :])
                nc.sync.dma_start(out=bo[c][:], in_=bo_d[sl, :])
            nc.sync.dma_start(out=bv_r[:], in_=bv_d[:])
            nc.sync.dma_start(out=gm[:], in_=gm_d[:])
            nc.sync.dma_start(out=bt[:], in_=bt_d[:])
            nc.sync.dma_start(out=e128[:], in_=e128_d[:])
            nc.vector.memset(ones_1x128[:], 1.0)
            nc.vector.memset(ones_128x1[:], 1.0)
            nc.vector.memset(ones_1xQ[:], 1.0)
            nc.vector.memset(epsT[:], LN_EPS)

            # preload the exp ACT table during projections so the first real
            # exp doesn't stall the attention pipeline for ~2.7us
            nc.vector.memset(warmup_in[:], 0.0)
            nc.scalar.activation(warmup_out[:], warmup_in[:], EXP)

            # ---- projections (all bf16, biases fused) ----
            with tc.tile_pool(name="pps", bufs=3, space="PSUM") as pps:
                # Q^T [2][128, 512] head-major partitions
                for mh in range(2):
                    ps = pps.tile([KB, Q], f32, tag="proj", name="proj")
                    for kc in range(2):
                        nc.tensor.matmul(
                            ps[:], wq[kc][:, mh * KB:(mh + 1) * KB], hq[kc][:],
                            start=(kc == 0), stop=(kc == 1),
                        )
                    nc.vector.tensor_scalar(
                        qTb[mh][:], ps[:], bq[mh][:], None, ADD
                    )
                # K^T [2][128, 4096]
                for mh in range(2):
                    for s in range(8):
                        ssl = slice(s * Q, (s + 1) * Q)
                        ps = pps.tile([KB, Q], f32, tag="proj", name="proj")
                        for kc in range(2):
                            nc.tensor.matmul(
                                ps[:], wk[kc][:, mh * KB:(mh + 1) * KB],
                                hT[kc][:, ssl],
                                start=(kc == 0), stop=(kc == 1),
                            )
                        nc.vector.tensor_scalar(
                            kT[mh][:, ssl], ps[:], bk[mh][:], None, ADD
                        )
                # V node-major [128, kb, h, 33] + bias via rank-1
                nc.vector.memset(vSB[:, :, :, DH], 1.0)
                for kb_i in range(NKB):
                    ksl = slice(kb_i * KB, (kb_i + 1) * KB)
                    psv = pps.tile([KB, H, DH], f32, tag="projv", name="projv")
                    for kc in range(2):
                        nc.tensor.matmul(
                            psv[:, :, :], hT[kc][:, ksl], wv[kc][:],
                            start=(kc == 0), stop=False,
                        )
                    nc.tensor.matmul(
                        psv[:, :, :], ones_1x128[:], bv_r[:],
                        start=False, stop=True,
                    )
                    nc.vector.tensor_copy(vSB[:, kb_i, :, 0:DH], psv[:, :, :])

            # ---- attention ----
            with ExitStack() as actx:
                sps = actx.enter_context(
                    tc.tile_pool(name="sps", bufs=3, space="PSUM")
                )
                ops = actx.enter_context(
                    tc.tile_pool(name="ops", bufs=2, space="PSUM")
                )
                spool = actx.enter_context(tc.tile_pool(name="spool", bufs=4))
                fpool = actx.enter_context(tc.tile_pool(name="fpool", bufs=3))
                npool = actx.enter_context(tc.tile_pool(name="npool", bufs=2))

                for mh in range(2):
                    # one oacc tile per head pair: partitions 0:33 head A
                    # (32 dims + z), 64:97 head B.
                    oacc = [
                        ops.tile([KB, Q], f32, tag="oacc", name="oacc")
                        for _ in range(2)
                    ]

                    def issue_pv(g):
                        t, pr, pf = g
                        first = (t == 0)
                        last = (t == NPAIR - 1)
                        for j in range(2):
                            kb_i = 2 * t + j
                            for hp in range(2):
                                h4 = 2 * pr + hp
                                h = 4 * mh + h4
                                nc.tensor.matmul(
                                    oacc[pr][64 * hp:64 * hp + DH + 1, :],
                                    vSB[:, kb_i, h, :],
                                    pf[hp][:, j * Q:(j + 1) * Q],
                                    start=(first and j == 0),
                                    stop=(last and j == 1),
                                    tile_position=(0, 64 * hp),
                                    skip_group_check=True,
                                )

                    # HAM warm-up: dependency-free burst of matmuls into the
                    # oacc region; the first real PV starts with start=True
                    # so the garbage is overwritten.
                    for w in range(WARM_START):
                        nc.tensor.matmul(
                            oacc[0][0:DH + 1, :], vSB[:, 0, 0, :], qTb[mh][:],
                            start=True, stop=True,
                            tile_position=(0, 0), skip_group_check=True,
                        )
                    # software pipeline: PE alternates QK(g) / PV(g-1) so it
                    # never stalls on the exp+mul chain of the current group.
                    prev = None
                    for t in range(NPAIR):
                        if WARM_PERIOD and mh == 0 and t == WARM_PERIOD:
                            # periodic dense matmul burst to re-flip HAM
                            bt_ps = sps.tile([KB, 2 * Q], f32, tag="sg",
                                             name="warm")
                            for w in range(WARM_LEN):
                                nc.tensor.matmul(
                                    bt_ps[:, 0:Q],
                                    kT[mh][0:32, 0:KB], qTb[mh][0:32, :],
                                    start=True, stop=True,
                                    tile_position=(0, 0),
                                    skip_group_check=True,
                                )
                        # one 1 MiB F transfer covers (mh, t) x 4 heads
                        fbig = fpool.tile([KB, 8 * Q], bf16, tag="ft",
                                          name="ft")
                        row = (mh * NPAIR + t) * KB
                        nc.sync.dma_start(
                            out=fbig[:], in_=F_d[row:row + KB, :]
                        )
                        for pr in range(2):      # head pairs (2p, 2p+1)
                            # QK: 2-head row-packed, j-outer for packing runs
                            sg = [
                                sps.tile([KB, 2 * Q], f32, tag="sg", name="sg")
                                for _ in range(2)
                            ]
                            # density boosters: dummy weight loads keep the
                            # PE array streaming (HAM warm) without touching
                            # PSUM; each real matmul reloads its own weights.
                            for w in range(WARM_BOOST):
                                nc.tensor.ldweights(
                                    kT[mh][:, 0:KB], tile_position=(0, 0),
                                )
                            for j in range(2):
                                kb_i = 2 * t + j
                                ksl = slice(kb_i * KB, (kb_i + 1) * KB)
                                for hp in range(2):
                                    h4 = 2 * pr + hp
                                    psl = slice(32 * h4, 32 * h4 + 32)
                                    nc.tensor.matmul(
                                        sg[hp][:, j * Q:(j + 1) * Q],
                                        kT[mh][psl, ksl],
                                        qTb[mh][psl, :],
                                        start=True, stop=True,
                                        tile_position=(32 * h4, 0),
                                    )
                            pf = [None, None]
                            for hp in range(2):
                                h4 = 2 * pr + hp
                                # exp (ACT) PSUM -> SBUF bf16
                                p0 = spool.tile(
                                    [KB, 2 * Q], bf16, tag="p0", name="p0"
                                )
                                nc.scalar.activation(
                                    p0[:], sg[hp][:], EXP, scale=SCALE
                                )
                                pf[hp] = spool.tile(
                                    [KB, 2 * Q], bf16, tag="pf", name="pf"
                                )
                                nc.vector.tensor_mul(
                                    pf[hp][:], p0[:],
                                    fbig[:, h4 * 2 * Q:(h4 + 1) * 2 * Q],
                                )
                            if prev is not None:
                                issue_pv(prev)
                            prev = (t, pr, pf)
                    issue_pv(prev)
                    prev = None
                    # ---- normalize: attn = oacc_num * (1/z) broadcast ----
                    # z rows: oacc[pr] partition 64*hp+32; gather to rows
                    # {0,32,64,96} of a [128, Q] tile, batch reciprocal.
                    zsb = npool.tile([KB, Q], f32, tag="zsb", name="zsb")
                    nc.vector.memset(zsb[:], 1.0)
                    for h4 in range(4):
                        pr, hp = h4 >> 1, h4 & 1
                        nc.vector.tensor_copy(
                            zsb[32 * h4:32 * h4 + 1, :],
                            oacc[pr][64 * hp + 32:64 * hp + 33, :],
                        )
                    rz = npool.tile([KB, Q], f32, tag="rz", name="rz")
                    nc.vector.reciprocal_approx_fast(rz[:], zsb[:])
                    rzb = npool.tile([KB, Q], bf16, tag="rzb", name="rzb")
                    nc.vector.tensor_copy(rzb[:], rz[:])
                    zbp = sps.tile([KB, Q], f32, tag="sg", name="zbp")
                    nc.tensor.matmul(
                        zbp[:], e128[:], rzb[:], start=True, stop=True
                    )
                    zbs = npool.tile([KB, Q], f32, tag="zbs", name="zbs")
                    nc.vector.tensor_copy(zbs[:], zbp[:])
                    for h4 in range(4):
                        pr, hp = h4 >> 1, h4 & 1
                        nc.vector.tensor_mul(
                            attnT[mh][32 * h4:32 * h4 + 32, :],
                            oacc[pr][64 * hp:64 * hp + 32, :],
                            zbs[32 * h4:32 * h4 + 32, :],
                        )

            # ---- output projection + residual + LayerNorm ----
            with ExitStack() as ectx:
                rps = ectx.enter_context(
                    tc.tile_pool(name="rps", bufs=1, space="PSUM")
                )
                epool = ectx.enter_context(tc.tile_pool(name="epool", bufs=2))
                out2 = [
                    epool.tile([KB, Q], f32, tag=f"out2_{c}", name=f"out2_{c}")
                    for c in range(2)
                ]
                for c in range(2):
                    op_ps = rps.tile([KB, Q], f32, tag="oproj", name="oproj")
                    for mh in range(2):
                        nc.tensor.matmul(
                            op_ps[:],
                            wo[mh][:, c * KB:(c + 1) * KB],
                            attnT[mh][:],
                            start=(mh == 0), stop=(mh == 1),
                        )
                    # out2 = (psum + bo) + x
                    nc.vector.scalar_tensor_tensor(
                        out2[c][:], op_ps[:], bo[c][:], xq[c][:],
                        op0=ADD, op1=ADD,
                    )
                # stats: mu, s2 via ones matmuls (f32)
                ones_f = epool.tile([KB, 1], f32, tag="onesf", name="onesf")
                nc.vector.memset(ones_f[:], 1.0)
                mu_ps = rps.tile([1, Q], f32, tag="mu", name="mu")
                for c in range(2):
                    nc.tensor.matmul(
                        mu_ps[:], ones_f[:], out2[c][:],
                        start=(c == 0), stop=(c == 1),
                        skip_group_check=True,
                    )
                s2_ps = rps.tile([1, Q], f32, tag="s2", name="s2")
                for c in range(2):
                    sq = epool.tile([KB, Q], f32, tag="sq", name="sq")
                    nc.vector.tensor_mul(sq[:], out2[c][:], out2[c][:])
                    nc.tensor.matmul(
                        s2_ps[:], ones_f[:], sq[:],
                        start=(c == 0), stop=(c == 1),
                        skip_group_check=True,
                    )
                mu = epool.tile([1, Q], f32, tag="mu_s", name="mu_s")
                nc.vector.tensor_scalar_mul(mu[:], mu_ps[:], 1.0 / D)
                m2 = epool.tile([1, Q], f32, tag="m2", name="m2")
                nc.vector.tensor_mul(m2[:], mu[:], mu[:])
                var = epool.tile([1, Q], f32, tag="var", name="var")
                nc.vector.scalar_tensor_tensor(
                    var[:], s2_ps[:], 1.0 / D, m2[:], op0=MULT, op1=SUB,
                )
                sd = epool.tile([1, Q], f32, tag="sd", name="sd")
                nc.scalar.activation(sd[:], var[:], SQRT, bias=epsT[:])
                rstd = epool.tile([1, Q], f32, tag="rstd", name="rstd")
                nc.vector.reciprocal_approx_fast(rstd[:], sd[:])
                # broadcast tiles via outer products:
                # c1 = gamma (x) rstd ; c2 = beta (x) 1 - gamma (x) (mu*rstd)
                rstd_b = epool.tile([1, Q], bf16, tag="rstdb", name="rstdb")
                nc.vector.tensor_copy(rstd_b[:], rstd[:])
                mr = epool.tile([1, Q], f32, tag="mr", name="mr")
                nc.vector.tensor_mul(mr[:], mu[:], rstd[:])
                mrn = epool.tile([1, Q], bf16, tag="mrn", name="mrn")
                nc.vector.tensor_scalar_mul(mrn[:], mr[:], -1.0)
                for c in range(2):
                    csl = slice(c * KB, (c + 1) * KB)
                    c1p = rps.tile([KB, Q], f32, tag="c1", name="c1")
                    nc.tensor.matmul(
                        c1p[:], gm[:, csl], rstd_b[:], start=True, stop=True
                    )
                    # c2 = gamma (x) (-mu*rstd) + beta (x) 1
                    c2p = rps.tile([KB, Q], f32, tag="c2", name="c2")
                    nc.tensor.matmul(
                        c2p[:], gm[:, csl], mrn[:], start=True, stop=False
                    )
                    nc.tensor.matmul(
                        c2p[:], bt[:, csl], ones_1xQ[:],
                        start=False, stop=True,
                    )
                    t1 = epool.tile([KB, Q], f32, tag="t1", name="t1")
                    nc.vector.tensor_mul(t1[:], out2[c][:], c1p[:])
                    y = epool.tile([KB, Q], f32, tag="y", name="y")
                    nc.vector.tensor_add(y[:], t1[:], c2p[:])
                    nc.sync.dma_start(out=outT[csl, :], in_=y[:])

    nc.compile()
    return nc


def _prep_F(q_idx, k_idx, bias_eh):
    """Dense multiplicative bias F = exp(scattered bias), per core.

    Row-block order matches kernel consumption: [mh, t, h4, partition]."""
    key = q_idx.astype(np.int64) * N + k_idx.astype(np.int64)
    uk, inv = np.unique(key, return_inverse=True)
    acc = np.zeros((len(uk), H), np.float32)
    np.add.at(acc, inv, bias_eh)
    uq = (uk // N).astype(np.int32)
    ukey = (uk % N).astype(np.int32)
    vals16 = np.exp(acc).astype(ml_dtypes.bfloat16).view(np.uint16)

    Fs = []
    for i in range(NCORES):
        sel = (uq >> 9) == i
        q = uq[sel] & (Q - 1)
        k = ukey[sel]
        v = vals16[sel]
        t = k >> 8
        j = (k >> 7) & 1
        p = k & (KB - 1)
        col = j * Q + q
        F16 = np.full((2, NPAIR, KB, 4, 2 * Q), 0x3F80, np.uint16)
        for h in range(H):
            F16[h >> 2, t, p, h & 3, col] = v[:, h]
        Fs.append(
            np.ascontiguousarray(F16.reshape(2 * NPAIR * KB, 4 * 2 * Q))
            .view(ml_dtypes.bfloat16)
        )
    return Fs


def kernel(**inputs):
    global LAST_RESULTS, _PROG
    x = np.asarray(inputs["x"], np.float32)
    pos = np.asarray(inputs["pos_encoding"], np.float32)
    ei = np.asarray(inputs["edge_index"])
    et = np.asarray(inputs["edge_types"])
    emb = np.asarray(inputs["edge_emb"], np.float32)
    W = {k: np.asarray(inputs[k], np.float32) for k in ("Wq", "Wk", "Wv", "Wo")}
    b = {k: np.asarray(inputs[k], np.float32).reshape(-1)
         for k in ("bq", "bk", "bv", "bo", "gamma", "beta")}

    bias_eh = emb[et]  # [E, H]
    Fs = _prep_F(ei[0], ei[1], bias_eh)

    pkey = (WARM_START, WARM_BOOST, WARM_PERIOD, WARM_LEN)
    if pkey not in _PROG:
        _PROG[pkey] = build_program()
    nc = _PROG[pkey]

    h = (x + pos).astype(np.float32)
    hT = np.ascontiguousarray(h.T.astype(ml_dtypes.bfloat16))
    xT = np.ascontiguousarray(x.T)
    Wb = {k: np.ascontiguousarray(w.astype(ml_dtypes.bfloat16))
          for k, w in W.items()}
    col = lambda a: np.ascontiguousarray(a.reshape(D, 1))
    row16 = lambda a: np.ascontiguousarray(
        a.reshape(1, D).astype(ml_dtypes.bfloat16)
    )
    e128 = np.zeros((KB, KB), np.float32)
    for h4 in range(4):
        e128[32 * h4, 32 * h4:32 * h4 + 32] = 1.0
    e128 = np.ascontiguousarray(e128.astype(ml_dtypes.bfloat16))

    in_maps = []
    for i in range(NCORES):
        sl = slice(i * Q, (i + 1) * Q)
        in_maps.append({
            "hT": hT,
            "hqT": np.ascontiguousarray(hT[:, sl]),
            "xqT": np.ascontiguousarray(xT[:, sl]),
            "Wq": Wb["Wq"], "Wk": Wb["Wk"], "Wv": Wb["Wv"], "Wo": Wb["Wo"],
            "bq": col(b["bq"]), "bk": col(b["bk"]), "bo": col(b["bo"]),
            "bv": row16(b["bv"]), "gm": row16(b["gamma"]),
            "bt": row16(b["beta"]), "e128": e128,
            "F": Fs[i],
        })

    trace = os.environ.get("BASS_KERNEL_TRACE", "0") == "1"
    try:
        res = run_bass_kernel_spmd(
            nc, in_maps, list(range(NCORES)), trace=trace
        )
    except Exception:
        if not trace:
            raise
        res = run_bass_kernel_spmd(nc, in_maps, list(range(NCORES)))
    LAST_RESULTS = res

    out = np.empty((N, D), np.float32)
    for i in range(NCORES):
        out[i * Q:(i + 1) * Q, :] = np.asarray(
            res.results[i]["outT"], np.float32
        ).T
    return out


# revision 35
# speedup vs baseline: 1.6733x; 1.2801x over previous
"""Graphormer layer (LocalSubgraphEncoder) Trainium2 Bass kernel, v2.

Sharding: node-parallel over 8 cores. Core i computes the full layer output
for query nodes [512*i, 512*i+512): all 8 heads of attention over all 4096
key nodes, edge-type bias, softmax, output projection, residual, LayerNorm.
No cross-core communication; host concatenates row slices.

v2 design (from perfetto analysis of v1: PE saturated by unpacked K=32
matmuls, GPSIMD dense local_scatter, STT stuck in 1x mode):
 - all matmuls bf16; 2-head row-packing for QK (tile_position row groups)
   and 2-head column-packing for PV / denominator matmuls.
 - scores layout S^T [keys(part), queries(free)]: softmax denominator z
   comes from a packed ones-vector matmul into a shared PSUM bank.
 - edge bias applied multiplicatively AFTER exp: P = exp(S) * F where
   F = exp(scattered bias) is precomputed DENSE on the host and streamed
   from HBM (33.5 MB/core) -> one 2x-mode DVE tensor_tensor per tile;
   GPSIMD does nothing.
 - ACT (ScalarE) does exclusively the exp drain PSUM->SBUF bf16 in
   [128,1024] tiles: the ~128 us floor every design shares.
 - biases fused into DVE copies (per-partition scalar AP) or rank-1 PE
   matmuls; LayerNorm scale/shift via outer-product matmuls.
"""
import os
import sys
import math
import numpy as np

sys.path.insert(0, "/opt/trn_rl_repo")
import ml_dtypes  # noqa: E402
from concourse import bacc, bass, mybir, tile  # noqa: E402
from concourse.bass_utils import run_bass_kernel_spmd  # noqa: E402

N, D, H, E, NT = 4096, 256, 8, 131072, 16
DH = D // H            # 32
NCORES = 8
Q = N // NCORES        # 512 query nodes per core
KB = 128               # key-node block (partition dim)
NKB = N // KB          # 32
NPAIR = NKB // 2       # 16 (two key-blocks per [128,1024] score tile)
LN_EPS = 1e-5
SCALE = 1.0 / math.sqrt(DH)

f32 = mybir.dt.float32
bf16 = mybir.dt.bfloat16
EXP = mybir.ActivationFunctionType.Exp
SQRT = mybir.ActivationFunctionType.Sqrt
ADD = mybir.AluOpType.add
MULT = mybir.AluOpType.mult
SUB = mybir.AluOpType.subtract

_PROG = {}
LAST_RESULTS = None

WARM_START = int(os.environ.get("WARM_START", "0"))
WARM_BOOST = int(os.environ.get("WARM_BOOST", "0"))
WARM_PERIOD = int(os.environ.get("WARM_PERIOD", "0"))
WARM_LEN = int(os.environ.get("WARM_LEN", "8"))


def build_program():
    nc = bacc.Bacc(
        "TRN2", target_bir_lowering=False, debug=False, num_devices=NCORES
    )

    def din(name, shape, dt):
        return nc.dram_tensor(name, shape, dt, kind="ExternalInput").ap()

    hT_d = din("hT", [D, N], bf16)          # (x + pos)^T
    xqT_d = din("xqT", [D, Q], f32)         # x^T core slice (residual)
    Wq_d = din("Wq", [D, D], bf16)
    Wk_d = din("Wk", [D, D], bf16)
    Wv_d = din("Wv", [D, D], bf16)
    Wo_d = din("Wo", [D, D], bf16)
    bq_d = din("bq", [D, 1], f32)
    bk_d = din("bk", [D, 1], f32)
    bo_d = din("bo", [D, 1], f32)
    bv_d = din("bv", [1, D], bf16)
    gm_d = din("gm", [1, D], bf16)          # gamma row
    bt_d = din("bt", [1, D], bf16)          # beta row
    e128_d = din("e128", [KB, KB], bf16)    # block-broadcast matrix
    # dense exp(bias): row = (mh, t, partition), col = (h4, j, q)
    F_d = din("F", [2 * NPAIR * KB, 4 * 2 * Q], bf16)
    outT = nc.dram_tensor("outT", [D, Q], f32, kind="ExternalOutput").ap()

    hqT_d = din("hqT", [D, Q], bf16)        # h^T core query slice

    with tile.TileContext(nc) as tc:
        from contextlib import ExitStack

        with ExitStack() as ctx:
            cpool = ctx.enter_context(tc.tile_pool(name="consts", bufs=1))

            def ctile(shape, dt, tag):
                return cpool.tile(shape, dt, tag=tag, name=tag)

            # persistent SBUF residents
            hT = [ctile([KB, N], bf16, f"hT{c}") for c in range(2)]
            hq = [ctile([KB, Q], bf16, f"hq{c}") for c in range(2)]
            xq = [ctile([KB, Q], f32, f"xq{c}") for c in range(2)]
            wq = [ctile([KB, D], bf16, f"wq{c}") for c in range(2)]
            wk = [ctile([KB, D], bf16, f"wk{c}") for c in range(2)]
            wv = [ctile([KB, D], bf16, f"wv{c}") for c in range(2)]
            wo = [ctile([KB, D], bf16, f"wo{c}") for c in range(2)]
            bq = [ctile([KB, 1], f32, f"bq{c}") for c in range(2)]
            bk = [ctile([KB, 1], f32, f"bk{c}") for c in range(2)]
            bo = [ctile([KB, 1], f32, f"bo{c}") for c in range(2)]
            bv_r = ctile([1, D], bf16, "bv_r")
            gm = ctile([1, D], bf16, "gm")
            bt = ctile([1, D], bf16, "bt")
            e128 = ctile([KB, KB], bf16, "e128")
            kT = [ctile([KB, N], bf16, f"kT{c}") for c in range(2)]
            qTb = [ctile([KB, Q], bf16, f"qTb{c}") for c in range(2)]
            # V with ones column: [key, kb, h, 32 dims + 1 one]
            vSB = ctile([KB, NKB, H, DH + 1], bf16, "vSB")
            attnT = [ctile([KB, Q], bf16, f"attnT{c}") for c in range(2)]
            ones_1x128 = ctile([1, KB], bf16, "o1x128")
            ones_128x1 = ctile([KB, 1], bf16, "o128x1")
            ones_1xQ = ctile([1, Q], bf16, "o1xQ")
            epsT = ctile([1, 1], f32, "epsT")
            warmup_in = ctile([1, 32], f32, "warmup_in")
            warmup_out = ctile([1, 32], bf16, "warmup_out")

            # ---- loads ----
            for c in range(2):
                sl = slice(c * KB, (c + 1) * KB)
                nc.sync.dma_start(out=hT[c][:], in_=hT_d[sl, :])
                nc.sync.dma_start(out=hq[c][:], in_=hqT_d[sl, :])
                nc.sync.dma_start(out=xq[c][:], in_=xqT_d[sl, :])
                nc.sync.dma_start(out=wq[c][:], in_=Wq_d[sl, :])
                nc.sync.dma_start(out=wk[c][:], in_=Wk_d[sl, :])
                nc.sync.dma_start(out=wv[c][:], in_=Wv_d[sl, :])
                nc.sync.dma_start(out=wo[c][:], in_=Wo_d[sl, :])
                nc.sync.dma_start(out=bq[c][:], in_=bq_d[sl, :])
                nc.sync.dma_start(out=bk[c][:], in_=bk_d[sl, :])
                nc.sync.dma_start(out=bo[c][:], in_=bo_d[sl, :])
            nc.sync.dma_start(out=bv_r[:], in_=bv_d[:])
            nc.sync.dma_start(out=gm[:], in_=gm_d[:])
            nc.sync.dma_start(out=bt[:], in_=bt_d[:])
            nc.sync.dma_start(out=e128[:], in_=e128_d[:])
            nc.vector.memset(ones_1x128[:], 1.0)
            nc.vector.memset(ones_128x1[:], 1.0)
            nc.vector.memset(ones_1xQ[:], 1.0)
            nc.vector.memset(epsT[:], LN_EPS)

            # preload the exp ACT table during projections so the first real
            # exp doesn't stall the attention pipeline for ~2.7us
            nc.vector.memset(warmup_in[:], 0.0)
            nc.scalar.activation(warmup_out[:], warmup_in[:], EXP)

            # ---- projections (all bf16, biases fused) ----
            with tc.tile_pool(name="pps", bufs=3, space="PSUM") as pps:
                # Q^T [2][128, 512] head-major partitions
                for mh in range(2):
                    ps = pps.tile([KB, Q], f32, tag="proj", name="proj")
                    for kc in range(2):
                        nc.tensor.matmul(
                            ps[:], wq[kc][:, mh * KB:(mh + 1) * KB], hq[kc][:],
                            start=(kc == 0), stop=(kc == 1),
                        )
                    nc.vector.tensor_scalar(
                        qTb[mh][:], ps[:], bq[mh][:], None, ADD
                    )
                # K^T [2][128, 4096]
                for mh in range(2):
                    for s in range(8):
                        ssl = slice(s * Q, (s + 1) * Q)
                        ps = pps.tile([KB, Q], f32, tag="proj", name="proj")
                        for kc in range(2):
                            nc.tensor.matmul(
                                ps[:], wk[kc][:, mh * KB:(mh + 1) * KB],
                                hT[kc][:, ssl],
                                start=(kc == 0), stop=(kc == 1),
                            )
                        nc.vector.tensor_scalar(
                            kT[mh][:, ssl], ps[:], bk[mh][:], None, ADD
                        )
                # V node-major [128, kb, h, 33] + bias via rank-1
                nc.vector.memset(vSB[:, :, :, DH], 1.0)
                for kb_i in range(NKB):
                    ksl = slice(kb_i * KB, (kb_i + 1) * KB)
                    psv = pps.tile([KB, H, DH], f32, tag="projv", name="projv")
                    for kc in range(2):
                        nc.tensor.matmul(
                            psv[:, :, :], hT[kc][:, ksl], wv[kc][:],
                            start=(kc == 0), stop=False,
                        )
                    nc.tensor.matmul(
                        psv[:, :, :], ones_1x128[:], bv_r[:],
                        start=False, stop=True,
                    )
                    nc.vector.tensor_copy(vSB[:, kb_i, :, 0:DH], psv[:, :, :])

            # ---- attention ----
            with ExitStack() as actx:
                sps = actx.enter_context(
                    tc.tile_pool(name="sps", bufs=3, space="PSUM")
                )
                ops = actx.enter_context(
                    tc.tile_pool(name="ops", bufs=2, space="PSUM")
                )
                spool = actx.enter_context(tc.tile_pool(name="spool", bufs=4))
                fpool = actx.enter_context(tc.tile_pool(name="fpool", bufs=3))
                npool = actx.enter_context(tc.tile_pool(name="npool", bufs=2))

                for mh in range(2):
                    # one oacc tile per head pair: partitions 0:33 head A
                    # (32 dims + z), 64:97 head B.
                    oacc = [
                        ops.tile([KB, Q], f32, tag="oacc", name="oacc")
                        for _ in range(2)
                    ]

                    def issue_pv(g):
                        t, pr, pf = g
                        first = (t == 0)
                        last = (t == NPAIR - 1)
                        for j in range(2):
                            kb_i = 2 * t + j
                            for hp in range(2):
                                h4 = 2 * pr + hp
                                h = 4 * mh + h4
                                nc.tensor.matmul(
                                    oacc[pr][64 * hp:64 * hp + DH + 1, :],
                                    vSB[:, kb_i, h, :],
                                    pf[j][:, hp * Q:(hp + 1) * Q],
                                    start=(first and j == 0),
                                    stop=(last and j == 1),
                                    tile_position=(0, 64 * hp),
                                    skip_group_check=True,
                                )

                    # HAM warm-up: dependency-free burst of matmuls into the
                    # oacc region; the first real PV starts with start=True
                    # so the garbage is overwritten.
                    for w in range(WARM_START):
                        nc.tensor.matmul(
                            oacc[0][0:DH + 1, :], vSB[:, 0, 0, :], qTb[mh][:],
                            start=True, stop=True,
                            tile_position=(0, 0), skip_group_check=True,
                        )
                    # software pipeline: PE alternates QK(g) / PV(g-1) so it
                    # never stalls on the exp+mul chain of the current group.
                    prev = None
                    for t in range(NPAIR):
                        if WARM_PERIOD and mh == 0 and t == WARM_PERIOD:
                            # periodic dense matmul burst to re-flip HAM
                            bt_ps = sps.tile([KB, 2 * Q], f32, tag="sg",
                                             name="warm")
                            for w in range(WARM_LEN):
                                nc.tensor.matmul(
                                    bt_ps[:, 0:Q],
                                    kT[mh][0:32, 0:KB], qTb[mh][0:32, :],
                                    start=True, stop=True,
                                    tile_position=(0, 0),
                                    skip_group_check=True,
                                )
                        # one 1 MiB F transfer covers (mh, t) x 4 heads
                        fbig = fpool.tile([KB, 8 * Q], bf16, tag="ft",
                                          name="ft")
                        row = (mh * NPAIR + t) * KB
                        nc.sync.dma_start(
                            out=fbig[:], in_=F_d[row:row + KB, :]
                        )
                        for pr in range(2):      # head pairs (2p, 2p+1)
                            # score tiles pair TWO HEADS at the same j so the
                            # two QK matmuls of a tile land on different PE
                            # row groups and run concurrently (row packing).
                            sg = [
                                sps.tile([KB, 2 * Q], f32, tag="sg", name="sg")
                                for _ in range(2)
                            ]
                            # density boosters (optional): dummy weight loads
                            for w in range(WARM_BOOST):
                                nc.tensor.ldweights(
                                    kT[mh][:, 0:KB], tile_position=(0, 0),
                                )
                            for j in range(2):
                                kb_i = 2 * t + j
                                ksl = slice(kb_i * KB, (kb_i + 1) * KB)
                                for hp in range(2):
                                    h4 = 2 * pr + hp
                                    psl = slice(32 * h4, 32 * h4 + 32)
                                    nc.tensor.matmul(
                                        sg[j][:, hp * Q:(hp + 1) * Q],
                                        kT[mh][psl, ksl],
                                        qTb[mh][psl, :],
                                        start=True, stop=True,
                                        tile_position=(32 * h4, 0),
                                    )
                            pf = [None, None]
                            for j in range(2):
                                # exp (ACT) PSUM -> SBUF bf16
                                p0 = spool.tile(
                                    [KB, 2 * Q], bf16, tag="p0", name="p0"
                                )
                                nc.scalar.activation(
                                    p0[:], sg[j][:], EXP, scale=SCALE
                                )
                                pf[j] = spool.tile(
                                    [KB, 2 * Q], bf16, tag="pf", name="pf"
                                )
                                nc.vector.tensor_mul(
                                    pf[j][:], p0[:],
                                    fbig[:, (pr * 2 + j) * 2 * Q:
                                         (pr * 2 + j + 1) * 2 * Q],
                                )
                            if prev is not None:
                                issue_pv(prev)
                            prev = (t, pr, pf)
                    issue_pv(prev)
                    prev = None
                    # ---- normalize: attn = oacc_num * (1/z) broadcast ----
                    # z rows: oacc[pr] partition 64*hp+32; gather to rows
                    # {0,32,64,96} of a [128, Q] tile, batch reciprocal.
                    zsb = npool.tile([KB, Q], f32, tag="zsb", name="zsb")
                    nc.vector.memset(zsb[:], 1.0)
                    for h4 in range(4):
                        pr, hp = h4 >> 1, h4 & 1
                        nc.vector.tensor_copy(
                            zsb[32 * h4:32 * h4 + 1, :],
                            oacc[pr][64 * hp + 32:64 * hp + 33, :],
                        )
                    rz = npool.tile([KB, Q], f32, tag="rz", name="rz")
                    nc.vector.reciprocal_approx_fast(rz[:], zsb[:])
                    rzb = npool.tile([KB, Q], bf16, tag="rzb", name="rzb")
                    nc.vector.tensor_copy(rzb[:], rz[:])
                    zbp = sps.tile([KB, Q], f32, tag="sg", name="zbp")
                    nc.tensor.matmul(
                        zbp[:], e128[:], rzb[:], start=True, stop=True
                    )
                    zbs = npool.tile([KB, Q], f32, tag="zbs", name="zbs")
                    nc.vector.tensor_copy(zbs[:], zbp[:])
                    for h4 in range(4):
                        pr, hp = h4 >> 1, h4 & 1
                        nc.vector.tensor_mul(
                            attnT[mh][32 * h4:32 * h4 + 32, :],
                            oacc[pr][64 * hp:64 * hp + 32, :],
                            zbs[32 * h4:32 * h4 + 32, :],
                        )

            # ---- output projection + residual + LayerNorm ----
            with ExitStack() as ectx:
                rps = ectx.enter_context(
                    tc.tile_pool(name="rps", bufs=1, space="PSUM")
                )
                epool = ectx.enter_context(tc.tile_pool(name="epool", bufs=2))
                out2 = [
                    epool.tile([KB, Q], f32, tag=f"out2_{c}", name=f"out2_{c}")
                    for c in range(2)
                ]
                for c in range(2):
                    op_ps = rps.tile([KB, Q], f32, tag="oproj", name="oproj")
                    for mh in range(2):
                        nc.tensor.matmul(
                            op_ps[:],
                            wo[mh][:, c * KB:(c + 1) * KB],
                            attnT[mh][:],
                            start=(mh == 0), stop=(mh == 1),
                        )
                    # out2 = (psum + bo) + x
                    nc.vector.scalar_tensor_tensor(
                        out2[c][:], op_ps[:], bo[c][:], xq[c][:],
                        op0=ADD, op1=ADD,
                    )
                # stats: mu, s2 via ones matmuls (f32)
                ones_f = epool.tile([KB, 1], f32, tag="onesf", name="onesf")
                nc.vector.memset(ones_f[:], 1.0)
                mu_ps = rps.tile([1, Q], f32, tag="mu", name="mu")
                for c in range(2):
                    nc.tensor.matmul(
                        mu_ps[:], ones_f[:], out2[c][:],
                        start=(c == 0), stop=(c == 1),
                        skip_group_check=True,
                    )
                s2_ps = rps.tile([1, Q], f32, tag="s2", name="s2")
                for c in range(2):
                    sq = epool.tile([KB, Q], f32, tag="sq", name="sq")
                    nc.vector.tensor_mul(sq[:], out2[c][:], out2[c][:])
                    nc.tensor.matmul(
                        s2_ps[:], ones_f[:], sq[:],
                        start=(c == 0), stop=(c == 1),
                        skip_group_check=True,
                    )
                mu = epool.tile([1, Q], f32, tag="mu_s", name="mu_s")
                nc.vector.tensor_scalar_mul(mu[:], mu_ps[:], 1.0 / D)
                m2 = epool.tile([1, Q], f32, tag="m2", name="m2")
                nc.vector.tensor_mul(m2[:], mu[:], mu[:])
                var = epool.tile([1, Q], f32, tag="var", name="var")
                nc.vector.scalar_tensor_tensor(
                    var[:], s2_ps[:], 1.0 / D, m2[:], op0=MULT, op1=SUB,
                )
                sd = epool.tile([1, Q], f32, tag="sd", name="sd")
                nc.scalar.activation(sd[:], var[:], SQRT, bias=epsT[:])
                rstd = epool.tile([1, Q], f32, tag="rstd", name="rstd")
                nc.vector.reciprocal_approx_fast(rstd[:], sd[:])
                # broadcast tiles via outer products:
                # c1 = gamma (x) rstd ; c2 = beta (x) 1 - gamma (x) (mu*rstd)
                rstd_b = epool.tile([1, Q], bf16, tag="rstdb", name="rstdb")
                nc.vector.tensor_copy(rstd_b[:], rstd[:])
                mr = epool.tile([1, Q], f32, tag="mr", name="mr")
                nc.vector.tensor_mul(mr[:], mu[:], rstd[:])
                mrn = epool.tile([1, Q], bf16, tag="mrn", name="mrn")
                nc.vector.tensor_scalar_mul(mrn[:], mr[:], -1.0)
                for c in range(2):
                    csl = slice(c * KB, (c + 1) * KB)
                    c1p = rps.tile([KB, Q], f32, tag="c1", name="c1")
                    nc.tensor.matmul(
                        c1p[:], gm[:, csl], rstd_b[:], start=True, stop=True
                    )
                    # c2 = gamma (x) (-mu*rstd) + beta (x) 1
                    c2p = rps.tile([KB, Q], f32, tag="c2", name="c2")
                    nc.tensor.matmul(
                        c2p[:], gm[:, csl], mrn[:], start=True, stop=False
                    )
                    nc.tensor.matmul(
                        c2p[:], bt[:, csl], ones_1xQ[:],
                        start=False, stop=True,
                    )
                    t1 = epool.tile([KB, Q], f32, tag="t1", name="t1")
                    nc.vector.tensor_mul(t1[:], out2[c][:], c1p[:])
                    y = epool.tile([KB, Q], f32, tag="y", name="y")
                    nc.vector.tensor_add(y[:], t1[:], c2p[:])
                    nc.sync.dma_start(out=outT[csl, :], in_=y[:])

    nc.compile()
    return nc


def _prep_F(q_idx, k_idx, bias_eh):
    """Dense multiplicative bias F = exp(scattered bias), per core.

    Row-block order matches kernel consumption: [mh, t, h4, partition]."""
    key = q_idx.astype(np.int64) * N + k_idx.astype(np.int64)
    uk, inv = np.unique(key, return_inverse=True)
    acc = np.zeros((len(uk), H), np.float32)
    np.add.at(acc, inv, bias_eh)
    uq = (uk // N).astype(np.int32)
    ukey = (uk % N).astype(np.int32)
    vals16 = np.exp(acc).astype(ml_dtypes.bfloat16).view(np.uint16)

    Fs = []
    for i in range(NCORES):
        sel = (uq >> 9) == i
        q = uq[sel] & (Q - 1)
        k = ukey[sel]
        v = vals16[sel]
        t = k >> 8
        j = (k >> 7) & 1
        p = k & (KB - 1)
        # cols ordered (pr, j, hp, q) to match paired score tiles
        F16 = np.full((2, NPAIR, KB, 2, 2, 2, Q), 0x3F80, np.uint16)
        for h in range(H):
            F16[h >> 2, t, p, (h & 3) >> 1, j, h & 1, q] = v[:, h]
        Fs.append(
            np.ascontiguousarray(F16.reshape(2 * NPAIR * KB, 4 * 2 * Q))
            .view(ml_dtypes.bfloat16)
        )
    return Fs


def kernel(**inputs):
    global LAST_RESULTS, _PROG
    x = np.asarray(inputs["x"], np.float32)
    pos = np.asarray(inputs["pos_encoding"], np.float32)
    ei = np.asarray(inputs["edge_index"])
    et = np.asarray(inputs["edge_types"])
    emb = np.asarray(inputs["edge_emb"], np.float32)
    W = {k: np.asarray(inputs[k], np.float32) for k in ("Wq", "Wk", "Wv", "Wo")}
    b = {k: np.asarray(inputs[k], np.float32).reshape(-1)
         for k in ("bq", "bk", "bv", "bo", "gamma", "beta")}

    bias_eh = emb[et]  # [E, H]
    Fs = _prep_F(ei[0], ei[1], bias_eh)

    pkey = (WARM_START, WARM_BOOST, WARM_PERIOD, WARM_LEN)
    if pkey not in _PROG:
        _PROG[pkey] = build_program()
    nc = _PROG[pkey]

    h = (x + pos).astype(np.float32)
    hT = np.ascontiguousarray(h.T.astype(ml_dtypes.bfloat16))
    xT = np.ascontiguousarray(x.T)
    Wb = {k: np.ascontiguousarray(w.astype(ml_dtypes.bfloat16))
          for k, w in W.items()}
    col = lambda a: np.ascontiguousarray(a.reshape(D, 1))
    row16 = lambda a: np.ascontiguousarray(
        a.reshape(1, D).astype(ml_dtypes.bfloat16)
    )
    e128 = np.zeros((KB, KB), np.float32)
    for h4 in range(4):
        e128[32 * h4, 32 * h4:32 * h4 + 32] = 1.0
    e128 = np.ascontiguousarray(e128.astype(ml_dtypes.bfloat16))

    in_maps = []
    for i in range(NCORES):
        sl = slice(i * Q, (i + 1) * Q)
        in_maps.append({
            "hT": hT,
            "hqT": np.ascontiguousarray(hT[:, sl]),
            "xqT": np.ascontiguousarray(xT[:, sl]),
            "Wq": Wb["Wq"], "Wk": Wb["Wk"], "Wv": Wb["Wv"], "Wo": Wb["Wo"],
            "bq": col(b["bq"]), "bk": col(b["bk"]), "bo": col(b["bo"]),
            "bv": row16(b["bv"]), "gm": row16(b["gamma"]),
            "bt": row16(b["beta"]), "e128": e128,
            "F": Fs[i],
        })

    trace = os.environ.get("BASS_KERNEL_TRACE", "0") == "1"
    try:
        res = run_bass_kernel_spmd(
            nc, in_maps, list(range(NCORES)), trace=trace
        )
    except Exception:
        if not trace:
            raise
        res = run_bass_kernel_spmd(nc, in_maps, list(range(NCORES)))
    LAST_RESULTS = res

    out = np.empty((N, D), np.float32)
    for i in range(NCORES):
        out[i * Q:(i + 1) * Q, :] = np.asarray(
            res.results[i]["outT"], np.float32
        ).T
    return out


# revision 39
# speedup vs baseline: 1.7839x; 1.0661x over previous
"""Graphormer layer (LocalSubgraphEncoder) Trainium2 Bass kernel, v2.

Sharding: node-parallel over 8 cores. Core i computes the full layer output
for query nodes [512*i, 512*i+512): all 8 heads of attention over all 4096
key nodes, edge-type bias, softmax, output projection, residual, LayerNorm.
No cross-core communication; host concatenates row slices.

v2 design (from perfetto analysis of v1: PE saturated by unpacked K=32
matmuls, GPSIMD dense local_scatter, STT stuck in 1x mode):
 - all matmuls bf16; 2-head row-packing for QK (tile_position row groups)
   and 2-head column-packing for PV / denominator matmuls.
 - scores layout S^T [keys(part), queries(free)]: softmax denominator z
   comes from a packed ones-vector matmul into a shared PSUM bank.
 - edge bias applied multiplicatively AFTER exp: P = exp(S) * F where
   F = exp(scattered bias) is precomputed DENSE on the host and streamed
   from HBM (33.5 MB/core) -> one 2x-mode DVE tensor_tensor per tile;
   GPSIMD does nothing.
 - ACT (ScalarE) does exclusively the exp drain PSUM->SBUF bf16 in
   [128,1024] tiles: the ~128 us floor every design shares.
 - biases fused into DVE copies (per-partition scalar AP) or rank-1 PE
   matmuls; LayerNorm scale/shift via outer-product matmuls.
"""
import os
import sys
import math
import numpy as np

sys.path.insert(0, "/opt/trn_rl_repo")
import ml_dtypes  # noqa: E402
from concourse import bacc, bass, mybir, tile  # noqa: E402
from concourse.bass_utils import run_bass_kernel_spmd  # noqa: E402

N, D, H, E, NT = 4096, 256, 8, 131072, 16
DH = D // H            # 32
NCORES = 8
Q = N // NCORES        # 512 query nodes per core
KB = 128               # key-node block (partition dim)
NKB = N // KB          # 32
NPAIR = NKB // 2       # 16 (two key-blocks per [128,1024] score tile)
LN_EPS = 1e-5
SCALE = 1.0 / math.sqrt(DH)

f32 = mybir.dt.float32
bf16 = mybir.dt.bfloat16
EXP = mybir.ActivationFunctionType.Exp
SQRT = mybir.ActivationFunctionType.Sqrt
ADD = mybir.AluOpType.add
MULT = mybir.AluOpType.mult
SUB = mybir.AluOpType.subtract

_PROG = {}
LAST_RESULTS = None

WARM_START = int(os.environ.get("WARM_START", "0"))
WARM_BOOST = int(os.environ.get("WARM_BOOST", "0"))
WARM_PERIOD = int(os.environ.get("WARM_PERIOD", "0"))
WARM_LEN = int(os.environ.get("WARM_LEN", "8"))


def build_program():
    nc = bacc.Bacc(
        "TRN2", target_bir_lowering=False, debug=False, num_devices=NCORES
    )

    def din(name, shape, dt):
        return nc.dram_tensor(name, shape, dt, kind="ExternalInput").ap()

    hT_d = din("hT", [D, N], bf16)          # (x + pos)^T
    xqT_d = din("xqT", [D, Q], f32)         # x^T core slice (residual)
    Wq_d = din("Wq", [D, D], bf16)
    Wk_d = din("Wk", [D, D], bf16)
    Wv_d = din("Wv", [D, D], bf16)
    Wo_d = din("Wo", [D, D], bf16)
    bq_d = din("bq", [D, 1], f32)
    bk_d = din("bk", [D, 1], f32)
    bo_d = din("bo", [D, 1], f32)
    bv_d = din("bv", [1, D], bf16)
    gm_d = din("gm", [1, D], bf16)          # gamma row
    bt_d = din("bt", [1, D], bf16)          # beta row
    e128_d = din("e128", [KB, KB], bf16)    # block-broadcast matrix
    # dense exp(bias): row = (mh, t, partition), col = (h4, j, q)
    F_d = din("F", [2 * NPAIR * KB, 4 * 2 * Q], bf16)
    outT = nc.dram_tensor("outT", [D, Q], f32, kind="ExternalOutput").ap()

    hqT_d = din("hqT", [D, Q], bf16)        # h^T core query slice

    with tile.TileContext(nc) as tc:
        from contextlib import ExitStack

        with ExitStack() as ctx:
            cpool = ctx.enter_context(tc.tile_pool(name="consts", bufs=1))

            def ctile(shape, dt, tag):
                return cpool.tile(shape, dt, tag=tag, name=tag)

            # persistent SBUF residents
            hT = [ctile([KB, N], bf16, f"hT{c}") for c in range(2)]
            hq = [ctile([KB, Q], bf16, f"hq{c}") for c in range(2)]
            xq = [ctile([KB, Q], f32, f"xq{c}") for c in range(2)]
            wq = [ctile([KB, D], bf16, f"wq{c}") for c in range(2)]
            wk = [ctile([KB, D], bf16, f"wk{c}") for c in range(2)]
            wv = [ctile([KB, D], bf16, f"wv{c}") for c in range(2)]
            wo = [ctile([KB, D], bf16, f"wo{c}") for c in range(2)]
            bq = [ctile([KB, 1], f32, f"bq{c}") for c in range(2)]
            bk = [ctile([KB, 1], f32, f"bk{c}") for c in range(2)]
            bo = [ctile([KB, 1], f32, f"bo{c}") for c in range(2)]
            bv_r = ctile([1, D], bf16, "bv_r")
            gm = ctile([1, D], bf16, "gm")
            bt = ctile([1, D], bf16, "bt")
            e128 = ctile([KB, KB], bf16, "e128")
            kT = [ctile([KB, N], bf16, f"kT{c}") for c in range(2)]
            qTb = [ctile([KB, Q], bf16, f"qTb{c}") for c in range(2)]
            # V with ones column: [key, kb, h, 32 dims + 1 one]
            vSB = ctile([KB, NKB, H, DH + 1], bf16, "vSB")
            attnT = [ctile([KB, Q], bf16, f"attnT{c}") for c in range(2)]
            ones_1x128 = ctile([1, KB], bf16, "o1x128")
            ones_128x1 = ctile([KB, 1], bf16, "o128x1")
            ones_1xQ = ctile([1, Q], bf16, "o1xQ")
            epsT = ctile([1, 1], f32, "epsT")
            warmup_in = ctile([1, 32], f32, "warmup_in")
            warmup_out = ctile([1, 32], bf16, "warmup_out")

            # ---- loads ----
            for c in range(2):
                sl = slice(c * KB, (c + 1) * KB)
                nc.sync.dma_start(out=hT[c][:], in_=hT_d[sl, :])
                nc.sync.dma_start(out=hq[c][:], in_=hqT_d[sl, :])
                nc.sync.dma_start(out=xq[c][:], in_=xqT_d[sl, :])
                nc.sync.dma_start(out=wq[c][:], in_=Wq_d[sl, :])
                nc.sync.dma_start(out=wk[c][:], in_=Wk_d[sl, :])
                nc.sync.dma_start(out=wv[c][:], in_=Wv_d[sl, :])
                nc.sync.dma_start(out=wo[c][:], in_=Wo_d[sl, :])
                nc.sync.dma_start(out=bq[c][:], in_=bq_d[sl, :])
                nc.sync.dma_start(out=bk[c][:], in_=bk_d[sl, :])
                nc.sync.dma_start(out=bo[c][:], in_=bo_d[sl, :])
            nc.sync.dma_start(out=bv_r[:], in_=bv_d[:])
            nc.sync.dma_start(out=gm[:], in_=gm_d[:])
            nc.sync.dma_start(out=bt[:], in_=bt_d[:])
            nc.sync.dma_start(out=e128[:], in_=e128_d[:])
            nc.vector.memset(ones_1x128[:], 1.0)
            nc.vector.memset(ones_128x1[:], 1.0)
            nc.vector.memset(ones_1xQ[:], 1.0)
            nc.vector.memset(epsT[:], LN_EPS)

            # preload the exp ACT table during projections so the first real
            # exp doesn't stall the attention pipeline for ~2.7us
            nc.vector.memset(warmup_in[:], 0.0)
            nc.scalar.activation(warmup_out[:], warmup_in[:], EXP)

            # ---- projections (all bf16, biases fused) ----
            with tc.tile_pool(name="pps", bufs=3, space="PSUM") as pps:
                # Q^T [2][128, 512] head-major partitions
                for mh in range(2):
                    ps = pps.tile([KB, Q], f32, tag="proj", name="proj")
                    for kc in range(2):
                        nc.tensor.matmul(
                            ps[:], wq[kc][:, mh * KB:(mh + 1) * KB], hq[kc][:],
                            start=(kc == 0), stop=(kc == 1),
                        )
                    nc.vector.tensor_scalar(
                        qTb[mh][:], ps[:], bq[mh][:], None, ADD
                    )
                # K^T [2][128, 4096]
                for mh in range(2):
                    for s in range(8):
                        ssl = slice(s * Q, (s + 1) * Q)
                        ps = pps.tile([KB, Q], f32, tag="proj", name="proj")
                        for kc in range(2):
                            nc.tensor.matmul(
                                ps[:], wk[kc][:, mh * KB:(mh + 1) * KB],
                                hT[kc][:, ssl],
                                start=(kc == 0), stop=(kc == 1),
                            )
                        nc.vector.tensor_scalar(
                            kT[mh][:, ssl], ps[:], bk[mh][:], None, ADD
                        )
                # V node-major [128, kb, h, 33] + bias via rank-1
                nc.vector.memset(vSB[:, :, :, DH], 1.0)
                for kb_i in range(NKB):
                    ksl = slice(kb_i * KB, (kb_i + 1) * KB)
                    psv = pps.tile([KB, H, DH], f32, tag="projv", name="projv")
                    for kc in range(2):
                        nc.tensor.matmul(
                            psv[:, :, :], hT[kc][:, ksl], wv[kc][:],
                            start=(kc == 0), stop=False,
                        )
                    nc.tensor.matmul(
                        psv[:, :, :], ones_1x128[:], bv_r[:],
                        start=False, stop=True,
                    )
                    nc.vector.tensor_copy(vSB[:, kb_i, :, 0:DH], psv[:, :, :])

            # ---- attention ----
            with ExitStack() as actx:
                sps = actx.enter_context(
                    tc.tile_pool(name="sps", bufs=3, space="PSUM")
                )
                ops = actx.enter_context(
                    tc.tile_pool(name="ops", bufs=2, space="PSUM")
                )
                spool = actx.enter_context(tc.tile_pool(name="spool", bufs=6))
                fpool = actx.enter_context(tc.tile_pool(name="fpool", bufs=4))
                npool = actx.enter_context(tc.tile_pool(name="npool", bufs=2))

                for mh in range(2):
                    # one oacc tile per head pair: partitions 0:33 head A
                    # (32 dims + z), 64:97 head B.
                    oacc = [
                        ops.tile([KB, Q], f32, tag="oacc", name="oacc")
                        for _ in range(2)
                    ]

                    def issue_pv(g):
                        t, pr, pf = g
                        first = (t == 0)
                        last = (t == NPAIR - 1)
                        for j in range(2):
                            kb_i = 2 * t + j
                            for hp in range(2):
                                h4 = 2 * pr + hp
                                h = 4 * mh + h4
                                nc.tensor.matmul(
                                    oacc[pr][64 * hp:64 * hp + DH + 1, :],
                                    vSB[:, kb_i, h, :],
                                    pf[j][:, hp * Q:(hp + 1) * Q],
                                    start=(first and j == 0),
                                    stop=(last and j == 1),
                                    tile_position=(0, 64 * hp),
                                    skip_group_check=True,
                                )

                    # HAM warm-up: dependency-free burst of matmuls into the
                    # oacc region; the first real PV starts with start=True
                    # so the garbage is overwritten.
                    for w in range(WARM_START):
                        nc.tensor.matmul(
                            oacc[0][0:DH + 1, :], vSB[:, 0, 0, :], qTb[mh][:],
                            start=True, stop=True,
                            tile_position=(0, 0), skip_group_check=True,
                        )
                    # software pipeline, lag 2: PV(g-2) issues BEFORE QK(g)
                    # so the in-order PE always has dependency-free work.
                    from collections import deque
                    pend = deque()
                    for t in range(NPAIR):
                        if WARM_PERIOD and mh == 0 and t == WARM_PERIOD:
                            # periodic dense matmul burst to re-flip HAM
                            bt_ps = sps.tile([KB, 2 * Q], f32, tag="sg",
                                             name="warm")
                            for w in range(WARM_LEN):
                                nc.tensor.matmul(
                                    bt_ps[:, 0:Q],
                                    kT[mh][0:32, 0:KB], qTb[mh][0:32, :],
                                    start=True, stop=True,
                                    tile_position=(0, 0),
                                    skip_group_check=True,
                                )
                        # one 1 MiB F transfer covers (mh, t) x 4 heads
                        fbig = fpool.tile([KB, 8 * Q], bf16, tag="ft",
                                          name="ft")
                        row = (mh * NPAIR + t) * KB
                        nc.sync.dma_start(
                            out=fbig[:], in_=F_d[row:row + KB, :]
                        )
                        for pr in range(2):      # head pairs (2p, 2p+1)
                            if len(pend) >= 2:
                                issue_pv(pend.popleft())
                            # score tiles pair TWO HEADS at the same j so the
                            # two QK matmuls of a tile land on different PE
                            # row groups and run concurrently (row packing).
                            sg = [
                                sps.tile([KB, 2 * Q], f32, tag="sg", name="sg")
                                for _ in range(2)
                            ]
                            # density boosters (optional): dummy weight loads
                            for w in range(WARM_BOOST):
                                nc.tensor.ldweights(
                                    kT[mh][:, 0:KB], tile_position=(0, 0),
                                )
                            for j in range(2):
                                kb_i = 2 * t + j
                                ksl = slice(kb_i * KB, (kb_i + 1) * KB)
                                for hp in range(2):
                                    h4 = 2 * pr + hp
                                    psl = slice(32 * h4, 32 * h4 + 32)
                                    nc.tensor.matmul(
                                        sg[j][:, hp * Q:(hp + 1) * Q],
                                        kT[mh][psl, ksl],
                                        qTb[mh][psl, :],
                                        start=True, stop=True,
                                        tile_position=(32 * h4, 0),
                                    )
                            pf = [None, None]
                            for j in range(2):
                                # exp (ACT) PSUM -> SBUF bf16
                                p0 = spool.tile(
                                    [KB, 2 * Q], bf16, tag="p0", name="p0"
                                )
                                nc.scalar.activation(
                                    p0[:], sg[j][:], EXP, scale=SCALE
                                )
                                pf[j] = spool.tile(
                                    [KB, 2 * Q], bf16, tag="pf", name="pf"
                                )
                                nc.vector.tensor_mul(
                                    pf[j][:], p0[:],
                                    fbig[:, (pr * 2 + j) * 2 * Q:
                                         (pr * 2 + j + 1) * 2 * Q],
                                )
                            pend.append((t, pr, pf))
                    while pend:
                        issue_pv(pend.popleft())
                    # ---- normalize: attn = oacc_num * (1/z) broadcast ----
                    # z rows: oacc[pr] partition 64*hp+32; gather to rows
                    # {0,32,64,96} of a [128, Q] tile, batch reciprocal.
                    zsb = npool.tile([KB, Q], f32, tag="zsb", name="zsb")
                    nc.vector.memset(zsb[:], 1.0)
                    for h4 in range(4):
                        pr, hp = h4 >> 1, h4 & 1
                        nc.vector.tensor_copy(
                            zsb[32 * h4:32 * h4 + 1, :],
                            oacc[pr][64 * hp + 32:64 * hp + 33, :],
                        )
                    rz = npool.tile([KB, Q], f32, tag="rz", name="rz")
                    nc.vector.reciprocal_approx_fast(rz[:], zsb[:])
                    rzb = npool.tile([KB, Q], bf16, tag="rzb", name="rzb")
                    nc.vector.tensor_copy(rzb[:], rz[:])
                    zbp = sps.tile([KB, Q], f32, tag="sg", name="zbp")
                    nc.tensor.matmul(
                        zbp[:], e128[:], rzb[:], start=True, stop=True
                    )
                    zbs = npool.tile([KB, Q], f32, tag="zbs", name="zbs")
                    nc.vector.tensor_copy(zbs[:], zbp[:])
                    for h4 in range(4):
                        pr, hp = h4 >> 1, h4 & 1
                        nc.vector.tensor_mul(
                            attnT[mh][32 * h4:32 * h4 + 32, :],
                            oacc[pr][64 * hp:64 * hp + 32, :],
                            zbs[32 * h4:32 * h4 + 32, :],
                        )

            # ---- output projection + residual + LayerNorm ----
            with ExitStack() as ectx:
                rps = ectx.enter_context(
                    tc.tile_pool(name="rps", bufs=1, space="PSUM")
                )
                epool = ectx.enter_context(tc.tile_pool(name="epool", bufs=2))
                out2 = [
                    epool.tile([KB, Q], f32, tag=f"out2_{c}", name=f"out2_{c}")
                    for c in range(2)
                ]
                for c in range(2):
                    op_ps = rps.tile([KB, Q], f32, tag="oproj", name="oproj")
                    for mh in range(2):
                        nc.tensor.matmul(
                            op_ps[:],
                            wo[mh][:, c * KB:(c + 1) * KB],
                            attnT[mh][:],
                            start=(mh == 0), stop=(mh == 1),
                        )
                    # out2 = (psum + bo) + x
                    nc.vector.scalar_tensor_tensor(
                        out2[c][:], op_ps[:], bo[c][:], xq[c][:],
                        op0=ADD, op1=ADD,
                    )
                # stats: mu, s2 via ones matmuls (f32)
                ones_f = epool.tile([KB, 1], f32, tag="onesf", name="onesf")
                nc.vector.memset(ones_f[:], 1.0)
                mu_ps = rps.tile([1, Q], f32, tag="mu", name="mu")
                for c in range(2):
                    nc.tensor.matmul(
                        mu_ps[:], ones_f[:], out2[c][:],
                        start=(c == 0), stop=(c == 1),
                        skip_group_check=True,
                    )
                s2_ps = rps.tile([1, Q], f32, tag="s2", name="s2")
                for c in range(2):
                    sq = epool.tile([KB, Q], f32, tag="sq", name="sq")
                    nc.vector.tensor_mul(sq[:], out2[c][:], out2[c][:])
                    nc.tensor.matmul(
                        s2_ps[:], ones_f[:], sq[:],
                        start=(c == 0), stop=(c == 1),
                        skip_group_check=True,
                    )
                mu = epool.tile([1, Q], f32, tag="mu_s", name="mu_s")
                nc.vector.tensor_scalar_mul(mu[:], mu_ps[:], 1.0 / D)
                m2 = epool.tile([1, Q], f32, tag="m2", name="m2")
                nc.vector.tensor_mul(m2[:], mu[:], mu[:])
                var = epool.tile([1, Q], f32, tag="var", name="var")
                nc.vector.scalar_tensor_tensor(
                    var[:], s2_ps[:], 1.0 / D, m2[:], op0=MULT, op1=SUB,
                )
                sd = epool.tile([1, Q], f32, tag="sd", name="sd")
                nc.scalar.activation(sd[:], var[:], SQRT, bias=epsT[:])
                rstd = epool.tile([1, Q], f32, tag="rstd", name="rstd")
                nc.vector.reciprocal_approx_fast(rstd[:], sd[:])
                # broadcast tiles via outer products:
                # c1 = gamma (x) rstd ; c2 = beta (x) 1 - gamma (x) (mu*rstd)
                rstd_b = epool.tile([1, Q], bf16, tag="rstdb", name="rstdb")
                nc.vector.tensor_copy(rstd_b[:], rstd[:])
                mr = epool.tile([1, Q], f32, tag="mr", name="mr")
                nc.vector.tensor_mul(mr[:], mu[:], rstd[:])
                mrn = epool.tile([1, Q], bf16, tag="mrn", name="mrn")
                nc.vector.tensor_scalar_mul(mrn[:], mr[:], -1.0)
                for c in range(2):
                    csl = slice(c * KB, (c + 1) * KB)
                    c1p = rps.tile([KB, Q], f32, tag="c1", name="c1")
                    nc.tensor.matmul(
                        c1p[:], gm[:, csl], rstd_b[:], start=True, stop=True
                    )
                    # c2 = gamma (x) (-mu*rstd) + beta (x) 1
                    c2p = rps.tile([KB, Q], f32, tag="c2", name="c2")
                    nc.tensor.matmul(
                        c2p[:], gm[:, csl], mrn[:], start=True, stop=False
                    )
                    nc.tensor.matmul(
                        c2p[:], bt[:, csl], ones_1xQ[:],
                        start=False, stop=True,
                    )
                    t1 = epool.tile([KB, Q], f32, tag="t1", name="t1")
                    nc.vector.tensor_mul(t1[:], out2[c][:], c1p[:])
                    y = epool.tile([KB, Q], f32, tag="y", name="y")
                    nc.vector.tensor_add(y[:], t1[:], c2p[:])
                    nc.sync.dma_start(out=outT[csl, :], in_=y[:])

    nc.compile()
    return nc


def _prep_F(q_idx, k_idx, bias_eh):
    """Dense multiplicative bias F = exp(scattered bias), per core.

    Row-block order matches kernel consumption: [mh, t, h4, partition]."""
    key = q_idx.astype(np.int64) * N + k_idx.astype(np.int64)
    uk, inv = np.unique(key, return_inverse=True)
    acc = np.zeros((len(uk), H), np.float32)
    np.add.at(acc, inv, bias_eh)
    uq = (uk // N).astype(np.int32)
    ukey = (uk % N).astype(np.int32)
    vals16 = np.exp(acc).astype(ml_dtypes.bfloat16).view(np.uint16)

    Fs = []
    for i in range(NCORES):
        sel = (uq >> 9) == i
        q = uq[sel] & (Q - 1)
        k = ukey[sel]
        v = vals16[sel]
        t = k >> 8
        j = (k >> 7) & 1
        p = k & (KB - 1)
        # cols ordered (pr, j, hp, q) to match paired score tiles
        F16 = np.full((2, NPAIR, KB, 2, 2, 2, Q), 0x3F80, np.uint16)
        for h in range(H):
            F16[h >> 2, t, p, (h & 3) >> 1, j, h & 1, q] = v[:, h]
        Fs.append(
            np.ascontiguousarray(F16.reshape(2 * NPAIR * KB, 4 * 2 * Q))
            .view(ml_dtypes.bfloat16)
        )
    return Fs


def kernel(**inputs):
    global LAST_RESULTS, _PROG
    x = np.asarray(inputs["x"], np.float32)
    pos = np.asarray(inputs["pos_encoding"], np.float32)
    ei = np.asarray(inputs["edge_index"])
    et = np.asarray(inputs["edge_types"])
    emb = np.asarray(inputs["edge_emb"], np.float32)
    W = {k: np.asarray(inputs[k], np.float32) for k in ("Wq", "Wk", "Wv", "Wo")}
    b = {k: np.asarray(inputs[k], np.float32).reshape(-1)
         for k in ("bq", "bk", "bv", "bo", "gamma", "beta")}

    bias_eh = emb[et]  # [E, H]
    Fs = _prep_F(ei[0], ei[1], bias_eh)

    pkey = (WARM_START, WARM_BOOST, WARM_PERIOD, WARM_LEN)
    if pkey not in _PROG:
        _PROG[pkey] = build_program()
    nc = _PROG[pkey]

    h = (x + pos).astype(np.float32)
    hT = np.ascontiguousarray(h.T.astype(ml_dtypes.bfloat16))
    xT = np.ascontiguousarray(x.T)
    Wb = {k: np.ascontiguousarray(w.astype(ml_dtypes.bfloat16))
          for k, w in W.items()}
    col = lambda a: np.ascontiguousarray(a.reshape(D, 1))
    row16 = lambda a: np.ascontiguousarray(
        a.reshape(1, D).astype(ml_dtypes.bfloat16)
    )
    e128 = np.zeros((KB, KB), np.float32)
    for h4 in range(4):
        e128[32 * h4, 32 * h4:32 * h4 + 32] = 1.0
    e128 = np.ascontiguousarray(e128.astype(ml_dtypes.bfloat16))

    in_maps = []
    for i in range(NCORES):
        sl = slice(i * Q, (i + 1) * Q)
        in_maps.append({
            "hT": hT,
            "hqT": np.ascontiguousarray(hT[:, sl]),
            "xqT": np.ascontiguousarray(xT[:, sl]),
            "Wq": Wb["Wq"], "Wk": Wb["Wk"], "Wv": Wb["Wv"], "Wo": Wb["Wo"],
            "bq": col(b["bq"]), "bk": col(b["bk"]), "bo": col(b["bo"]),
            "bv": row16(b["bv"]), "gm": row16(b["gamma"]),
            "bt": row16(b["beta"]), "e128": e128,
            "F": Fs[i],
        })

    trace = os.environ.get("BASS_KERNEL_TRACE", "0") == "1"
    try:
        res = run_bass_kernel_spmd(
            nc, in_maps, list(range(NCORES)), trace=trace
        )
    except Exception:
        if not trace:
            raise
        res = run_bass_kernel_spmd(nc, in_maps, list(range(NCORES)))
    LAST_RESULTS = res

    out = np.empty((N, D), np.float32)
    for i in range(NCORES):
        out[i * Q:(i + 1) * Q, :] = np.asarray(
            res.results[i]["outT"], np.float32
        ).T
    return out
